# revision 13
# baseline (speedup 1.0000x reference)
"""TP-8 decode attention kernel for TRN2 (Bass/Tile), bf16 compute.

Shards the 8 KV heads (2 q heads each) across 8 NeuronCores. Host
pre-casts weights and KV cache to bf16 (HBM traffic 86.5 -> 43.3 MB per
core) and lays every tensor out in the exact tile order the kernel
consumes, so all big DMAs are contiguous and >= 1 MiB.

Per core: qkv projection (lhsT = xT k-tiles [128,8], rhs = Wq bf16
streamed N=512), RoPE on DVE, PE transposes to head-major qT/kT, scores
with q stationary ([128,16] lhsT, K streamed N=512, all-batch rows with
per-batch extraction at the PSUM drain), fused softmax (exp in place
with accumulated sum, probs pre-scaled by 1/norm so the A.V drain is a
plain copy), A.V with per-batch M=2 probsT columns vs V tiles (N=256),
new-token fixup as one extra rank-8 matmul per batch, out-proj partial
(lhsT = A^T tiles [128,8]). Host sums the 8 partial outputs.
"""

import sys

sys.path.insert(0, "/opt/trn_rl_repo")

import numpy as np
import ml_dtypes

B, S, C = 8, 1, 4096
DIM = 3072
HQ, HKV, HD = 16, 8, 256
NCORES = 8
SCALE = HD ** (-0.5)
BF = ml_dtypes.bfloat16

# packed f32 constant-block column offsets
_FM, _CS, _MKV, _DUPA, _DUPB, _IDF, _ONES = 0, 4096, 4608, 4609, 4617, 4633, 4649
_CSTW = 4657


def build_bass():
    import concourse.bass as bass  # noqa: F401
    import concourse.mybir as mybir
    import concourse.tile as tile
    from concourse import bacc
    from contextlib import ExitStack

    f32 = mybir.dt.float32
    bf16 = mybir.dt.bfloat16
    Alu = mybir.AluOpType
    Act = mybir.ActivationFunctionType

    nc = bacc.Bacc("TRN2", target_bir_lowering=False, debug=False,
                   num_devices=NCORES)

    xT = nc.dram_tensor("xT", [128, 24 * B], bf16, kind="ExternalInput").ap()
    wq = nc.dram_tensor("wq", [6, 128, 4096], bf16, kind="ExternalInput").ap()
    kt = nc.dram_tensor("kt", [8, 128, 8192], bf16, kind="ExternalInput").ap()
    fmb = nc.dram_tensor("fmb", [16, C], bf16, kind="ExternalInput").ap()
    vt = nc.dram_tensor("vt", [B, 128, 8192], bf16, kind="ExternalInput").ap()
    wo = nc.dram_tensor("wo", [128, 4 * DIM], bf16, kind="ExternalInput").ap()
    cst = nc.dram_tensor("cst", [16, _CSTW], f32, kind="ExternalInput").ap()
    cmk = nc.dram_tensor("cmk", [128, 144], bf16, kind="ExternalInput").ap()
    y = nc.dram_tensor("y", [B, DIM], f32, kind="ExternalOutput").ap()

    with tile.TileContext(nc) as tc, ExitStack() as stk:
        sb = stk.enter_context(tc.tile_pool(name="sb", bufs=1))
        wqp = stk.enter_context(tc.tile_pool(name="wqp", bufs=2))
        kp = stk.enter_context(tc.tile_pool(name="kp", bufs=2))
        vp = stk.enter_context(tc.tile_pool(name="vp", bufs=4))
        tmp = stk.enter_context(tc.tile_pool(name="tmp", bufs=4))
        ps = stk.enter_context(tc.tile_pool(name="ps", bufs=8, space="PSUM"))

        # ---- early small DMAs ----
        xT_sb = sb.tile([128, 24 * B], bf16, tag="xT")
        nc.sync.dma_start(xT_sb[:], xT)
        cst_sb = sb.tile([16, _CSTW], f32, tag="cst")
        nc.sync.dma_start(cst_sb[:], cst)
        cmk_sb = sb.tile([128, 144], bf16, tag="cmk")
        nc.sync.dma_start(cmk_sb[:], cmk)
        fm_sb = sb.tile([16, C], bf16, tag="fm")
        nc.sync.dma_start(fm_sb[:], fmb)
        cmask = cmk_sb[:, 0:128]
        idb16 = cmk_sb[0:16, 128:144]
        idb8 = cmk_sb[0:8, 128:136]
        idb2 = cmk_sb[0:2, 128:130]

        fm = fm_sb[:]
        cosq = cst_sb[0:8, _CS:_CS + 128]
        sinq = cst_sb[0:8, _CS + 128:_CS + 256]
        cosk = cst_sb[0:8, _CS + 256:_CS + 384]
        sink = cst_sb[0:8, _CS + 384:_CS + 512]
        mkv = cst_sb[:, _MKV:_MKV + 1]
        dupA = cst_sb[:, _DUPA:_DUPA + 8]
        dupB = cst_sb[0:8, _DUPB:_DUPB + 16]
        idf = cst_sb[:, _IDF:_IDF + 16]
        ones18 = cst_sb[0:1, _ONES:_ONES + 8]

        # HAM warmth: tiny real matmuls (transpose-mode doesn't count as
        # PE-busy) to bridge DMA-wait gaps so the PE clock stays at 2.4 GHz
        dps = ps.tile([16, 128], f32, tag="ps", name="dps")

        def warm(n=2):
            for _ in range(n):
                nc.tensor.matmul(dps[:], cmask[:, 0:16], cmask[:, 0:128],
                                 start=True, stop=True)

        # ---- phase 1: qkv = x @ Wq_shard  -> psum [8, 1024] (2 banks) ----
        pq0 = ps.tile([8, 512], f32, tag="ps", name="pq0")
        pq1 = ps.tile([8, 512], f32, tag="ps", name="pq1")
        for g in range(6):
            wt = wqp.tile([128, 4096], bf16, tag="wq")
            nc.sync.dma_start(wt[:], wq[g])
            for sub in range(4):
                t = 4 * g + sub
                lhs = xT_sb[:, t * 8:(t + 1) * 8]
                nc.tensor.matmul(pq0[:], lhs, wt[:, sub * 1024:sub * 1024 + 512],
                                 start=(t == 0), stop=(t == 23))
                nc.tensor.matmul(pq1[:], lhs,
                                 wt[:, sub * 1024 + 512:sub * 1024 + 1024],
                                 start=(t == 0), stop=(t == 23))
            warm(3)

        # ---- rope (DVE) on [8, 128] slices; outputs bf16 ----
        qrope = sb.tile([8, 512], bf16, tag="qrope")   # cols (r, half, p)
        krope = sb.tile([8, 256], bf16, tag="krope")   # cols (half, p)
        vnew = sb.tile([8, 256], bf16, tag="vnew")

        def rope(c1, c2, cosa, sina, out1, out2):
            ta = tmp.tile([8, 128], f32, tag="rt", name="ta")
            tb = tmp.tile([8, 128], f32, tag="rt", name="tb")
            nc.vector.tensor_tensor(ta[:], c1, cosa, op=Alu.mult)
            nc.vector.tensor_tensor(tb[:], c2, sina, op=Alu.mult)
            nc.vector.tensor_tensor(out1, ta[:], tb[:], op=Alu.subtract)
            tc_ = tmp.tile([8, 128], f32, tag="rt", name="tc_")
            td = tmp.tile([8, 128], f32, tag="rt", name="td")
            nc.vector.tensor_tensor(tc_[:], c1, sina, op=Alu.mult)
            nc.vector.tensor_tensor(td[:], c2, cosa, op=Alu.mult)
            nc.vector.tensor_tensor(out2, tc_[:], td[:], op=Alu.add)

        for r in range(2):
            rope(pq0[:, r * 256:r * 256 + 128], pq0[:, r * 256 + 128:(r + 1) * 256],
                 cosq, sinq,
                 qrope[:, (2 * r) * 128:(2 * r) * 128 + 128],
                 qrope[:, (2 * r + 1) * 128:(2 * r + 1) * 128 + 128])
        rope(pq1[:, 0:128], pq1[:, 128:256], cosk, sink,
             krope[:, 0:128], krope[:, 128:256])
        nc.scalar.copy(vnew[:], pq1[:, 256:512])

        # ---- transposes to qT halves [128,16] (cols 2b+r) and kT [128,8] ----
        qTh = [sb.tile([128, 16], bf16, tag=f"qTh{h}", name=f"qTh{h}")
               for h in range(2)]
        knT = [sb.tile([128, 8], bf16, tag=f"knT{h}", name=f"knT{h}")
               for h in range(2)]
        for r in range(2):
            for h in range(2):
                pt = ps.tile([128, 8], bf16, tag="ps", name=f"ptq{r}{h}")
                nc.tensor.transpose(pt[:], qrope[:, (2 * r + h) * 128:
                                                  (2 * r + h + 1) * 128],
                                    idb8)
                dst = qTh[h][:].rearrange("p (b r) -> p r b", r=2)[:, r]
                nc.scalar.copy(dst, pt[:])
        for h in range(2):
            pt = ps.tile([128, 8], bf16, tag="ps", name=f"ptk{h}")
            nc.tensor.transpose(pt[:], krope[:, h * 128:(h + 1) * 128],
                                idb8)
            nc.scalar.copy(knT[h][:], pt[:])

        # ---- s_new[16,1]: q . k_new, diag extraction ----
        psn = ps.tile([16, 8], f32, tag="ps", name="psn")
        for h in range(2):
            nc.tensor.matmul(psn[:], qTh[h][:], knT[h][:],
                             start=(h == 0), stop=(h == 1))
        snm = sb.tile([16, 8], f32, tag="snm")
        nc.vector.tensor_tensor(snm[:], psn[:], dupA, op=Alu.mult)
        s_new = sb.tile([16, 1], f32, tag="snew")
        nc.vector.tensor_reduce(s_new[:], snm[:], axis=mybir.AxisListType.X,
                                op=Alu.add)
        nc.vector.tensor_scalar_add(s_new[:], s_new[:], mkv)

        # masked q: qThM[h][:, b*16+c] = qTh[h][:, c] if c in {2b, 2b+1} else 0
        qThM = [sb.tile([128, 128], bf16, tag=f"qThM{h}", name=f"qThM{h}")
                for h in range(2)]
        for h in range(2):
            for b in range(B):
                nc.vector.tensor_tensor(qThM[h][:, b * 16:(b + 1) * 16],
                                        qTh[h][:],
                                        cmask[:, b * 16:(b + 1) * 16],
                                        op=Alu.mult)

        # ---- phase 2: scores [16, 4096]; kt tiles are chunk-major so each
        # psum bank accumulates all 16 (b, h) matmuls then drains at once ----
        scores = sb.tile([16, C], f32, tag="scores")
        mparts = sb.tile([16, 8], f32, tag="mparts")
        for j in range(8):
            ktile = kp.tile([128, 8192], bf16, tag="k", name=f"k{j}")
            nc.sync.dma_start(ktile[:], kt[j])
            pch = ps.tile([16, 512], f32, tag="ps", name=f"sc{j}")
            for b in range(B):
                nc.tensor.matmul(pch[:], qThM[0][:, b * 16:(b + 1) * 16],
                                 ktile[:, b * 1024:b * 1024 + 512],
                                 start=(b == 0), stop=False)
                nc.tensor.matmul(pch[:], qThM[1][:, b * 16:(b + 1) * 16],
                                 ktile[:, b * 1024 + 512:(b + 1) * 1024],
                                 start=False, stop=(b == B - 1))
            warm(2)
            ssl = slice(j * 512, (j + 1) * 512)
            nc.vector.tensor_tensor(scores[:, ssl], pch[:], fm[:, ssl],
                                    op=Alu.add)
            nc.vector.tensor_reduce(mparts[:, j:j + 1], scores[:, ssl],
                                    axis=mybir.AxisListType.X, op=Alu.max)
        warm(16)

        # ---- softmax (new token via s_new; probs pre-scaled by 1/norm) ----
        m1 = sb.tile([16, 1], f32, tag="m1")
        nc.vector.tensor_reduce(m1[:], mparts[:], axis=mybir.AxisListType.X,
                                op=Alu.max)
        tmax = sb.tile([16, 1], f32, tag="tmax")
        nc.vector.tensor_tensor(tmax[:], m1[:], s_new[:], op=Alu.max)
        negmax = sb.tile([16, 1], f32, tag="negmax")
        nc.vector.tensor_scalar_mul(negmax[:], tmax[:], -1.0)
        sumz = sb.tile([16, 1], f32, tag="sumz")
        nc.scalar.activation(scores[:], scores[:], Act.Exp, bias=negmax[:],
                             accum_out=sumz[:])
        p_new = sb.tile([16, 1], f32, tag="pnew")
        nc.scalar.activation(p_new[:], s_new[:], Act.Exp, bias=negmax[:])
        norm = sb.tile([16, 1], f32, tag="norm")
        nc.vector.tensor_tensor(norm[:], sumz[:], p_new[:], op=Alu.add)
        rnorm = sb.tile([16, 1], f32, tag="rnorm")
        nc.vector.reciprocal(rnorm[:], norm[:])
        probs = sb.tile([16, C], bf16, tag="probs")
        nc.vector.tensor_scalar_mul(probs[:], scores[:], rnorm[:, 0:1])
        pnorm = sb.tile([16, 1], f32, tag="pnorm")
        nc.vector.tensor_tensor(pnorm[:], p_new[:], rnorm[:], op=Alu.mult)

        # probsT via PE transpose: 32 x [16,128] -> [128,16] bf16
        probsT = sb.tile([128, 32 * 16], bf16, tag="probsT")
        for ct in range(32):
            pt = ps.tile([128, 16], bf16, tag="ps", name=f"pt{ct}")
            nc.tensor.transpose(pt[:], probs[:, ct * 128:(ct + 1) * 128],
                                idb16)
            nc.scalar.copy(probsT[:, ct * 16:(ct + 1) * 16], pt[:])

        # selPT[8,16] = dupB * broadcast(pnorm^T): for new-token A.V term
        pnt = ps.tile([1, 16], f32, tag="ps", name="pnt")
        nc.tensor.transpose(pnt[:], pnorm[:], idf)
        pnT = sb.tile([1, 16], f32, tag="pnT")
        nc.scalar.copy(pnT[:], pnt[:])
        pb = ps.tile([8, 16], f32, tag="ps", name="pb")
        nc.tensor.matmul(pb[:], ones18, pnT[:], start=True, stop=True)
        selPT = sb.tile([8, 16], bf16, tag="selPT")
        nc.vector.tensor_tensor(selPT[:], pb[:], dupB, op=Alu.mult)

        # ---- phase 3: A = probs @ V per batch, M=2, N=256 ----
        wo_sb = sb.tile([128, 4 * DIM], bf16, tag="wo")
        aTall = sb.tile([128, 32], bf16, tag="aTall")  # cols (r, half, b)
        for b in range(B):
            vtile = vp.tile([128, 8192], bf16, tag="v", name=f"v{b}")
            nc.sync.dma_start(vtile[:], vt[b])
            if b == 3:
                nc.sync.dma_start(wo_sb[:], wo)
            pav = ps.tile([2, 256], f32, tag="ps", name=f"av{b}")
            for ct in range(32):
                nc.tensor.matmul(pav[:],
                                 probsT[:, ct * 16 + 2 * b:ct * 16 + 2 * b + 2],
                                 vtile[:, ct * 256:(ct + 1) * 256],
                                 start=(ct == 0), stop=False)
            nc.tensor.matmul(pav[:], selPT[:, 2 * b:2 * b + 2], vnew[:],
                             start=False, stop=True)
            warm(2)
            asb = tmp.tile([2, 256], bf16, tag="asb", name=f"asb{b}")
            nc.vector.tensor_copy(asb[:], pav[:])
            for h in range(2):
                pt2 = ps.tile([128, 2], bf16, tag="ps", name=f"pat{b}{h}")
                nc.tensor.transpose(pt2[:], asb[:, h * 128:(h + 1) * 128],
                                    idb2)
                dst = aTall[:].rearrange("p (r h b) -> p h b r", r=2, h=2,
                                         b=8)[:, h, b]
                nc.scalar.copy(dst, pt2[:])

        # ---- phase 4: y = A^T tiles @ Wout_shard ----
        y_sb = sb.tile([B, DIM], f32, tag="ysb")
        pys = [ps.tile([8, 512], f32, tag="ps", name=f"py{n}")
               for n in range(6)]
        for t in range(4):
            for nch in range(6):
                nc.tensor.matmul(pys[nch][:], aTall[:, t * 8:(t + 1) * 8],
                                 wo_sb[:, t * DIM + nch * 512:
                                       t * DIM + (nch + 1) * 512],
                                 start=(t == 0), stop=(t == 3))
        for nch in range(6):
            nc.vector.tensor_copy(y_sb[:, nch * 512:(nch + 1) * 512],
                                  pys[nch][:])
        nc.sync.dma_start(y, y_sb[:])

    nc.compile()
    return nc


_CACHED = {}


def _get_bass():
    if "nc" not in _CACHED:
        _CACHED["nc"] = build_bass()
    return _CACHED["nc"]


def _prep_inputs(x, freqs_cos, freqs_sin, kv, k_cache, v_cache, mask,
                 W_qkv, W_out):
    x2 = np.asarray(x, np.float32).reshape(B, DIM)
    xT192 = np.ascontiguousarray(
        x2.T.reshape(24, 128, B).transpose(1, 0, 2).reshape(128, 24 * B)
    ).astype(BF)
    cos = np.asarray(freqs_cos, np.float32)[0]
    sin = np.asarray(freqs_sin, np.float32)[0]
    kvp = int(np.asarray(kv).reshape(-1)[0])
    maskr = np.asarray(mask, np.float32)

    cst = np.zeros((16, _CSTW), np.float32)
    fmb = np.tile(maskr, (16, 1)).astype(np.float32)
    fmb[:, kvp] -= 1e30
    fmb = fmb.astype(BF)
    cs = np.concatenate([cos * SCALE, sin * SCALE, cos, sin])
    cst[0:8, _CS:_CS + 512] = np.tile(cs, (8, 1))
    cst[:, _MKV] = maskr[0, kvp]
    for b in range(B):
        cst[2 * b, _DUPA + b] = 1.0
        cst[2 * b + 1, _DUPA + b] = 1.0
        cst[b, _DUPB + 2 * b] = 1.0
        cst[b, _DUPB + 2 * b + 1] = 1.0
    cst[:, _IDF:_IDF + 16] = np.eye(16, dtype=np.float32)
    cst[0, _ONES:_ONES + 8] = 1.0
    cmk = np.zeros((128, 144), np.float32)
    for b in range(B):
        cmk[:, b * 16 + 2 * b] = 1.0
        cmk[:, b * 16 + 2 * b + 1] = 1.0
    cmk[0:16, 128:144] = np.eye(16, dtype=np.float32)
    cmk = cmk.astype(BF)

    KB = np.asarray(k_cache, np.float32).astype(BF)   # [B, C, HKV, HD]
    VB = np.asarray(v_cache, np.float32).astype(BF)
    WqB = np.asarray(W_qkv, np.float32).astype(BF)    # [DIM, 8192]
    WoB = np.asarray(W_out, np.float32).astype(BF)    # [4096, DIM]

    in_maps = []
    for m in range(NCORES):
        wq_shard = np.concatenate([
            WqB[:, 2 * m * HD:(2 * m + 2) * HD],
            WqB[:, HQ * HD + m * HD: HQ * HD + (m + 1) * HD],
            WqB[:, (HQ + HKV) * HD + m * HD: (HQ + HKV) * HD + (m + 1) * HD],
        ], axis=1)                                     # [3072, 1024]
        wq6 = np.ascontiguousarray(
            wq_shard.reshape(6, 4, 128, 1024).transpose(0, 2, 1, 3)
        ).reshape(6, 128, 4096)
        kts = np.ascontiguousarray(
            KB[:, :, m, :].reshape(B, 8, 512, 2, 128)
            .transpose(1, 4, 0, 3, 2)
        ).reshape(8, 128, 8192)
        vts = np.ascontiguousarray(
            VB[:, :, m, :].reshape(B, 32, 128, HD).transpose(0, 2, 1, 3)
        ).reshape(B, 128, 8192)
        wo4 = np.ascontiguousarray(
            WoB[2 * m * HD:(2 * m + 2) * HD, :].reshape(2, 2, 128, DIM)
            .transpose(2, 0, 1, 3)
        ).reshape(128, 4 * DIM)
        in_maps.append({
            "xT": xT192, "wq": wq6, "kt": kts, "vt": vts, "wo": wo4,
            "cst": cst, "cmk": cmk, "fmb": fmb,
        })
    return in_maps


def _run(inputs, trace=False):
    from concourse.bass_utils import run_bass_kernel_spmd
    nc = _get_bass()
    in_maps = _prep_inputs(**inputs)
    res = run_bass_kernel_spmd(nc, in_maps, core_ids=list(range(NCORES)),
                               trace=trace)
    parts = [r["y"] for r in res.results]
    out = np.sum(np.stack(parts, 0), 0, dtype=np.float32)
    return out.reshape(B, S, DIM), res


def kernel(**inputs):
    out, _ = _run(inputs, trace=False)
    return out


# revision 19
# speedup vs baseline: 1.2611x; 1.2611x over previous
"""TP-8 decode attention kernel for TRN2 (Bass/Tile), bf16 compute.

Shards the 8 KV heads (2 q heads each) across 8 NeuronCores. Host
pre-casts weights and KV cache to bf16 (HBM traffic 86.5 -> 43.3 MB per
core) and lays every tensor out in the exact tile order the kernel
consumes, so all big DMAs are contiguous and >= 1 MiB.

Per core: qkv projection (lhsT = xT k-tiles [128,8], rhs = Wq bf16
streamed N=512), RoPE on DVE, PE transposes to head-major qT/kT, scores
with q stationary ([128,16] lhsT, K streamed N=512, all-batch rows with
per-batch extraction at the PSUM drain), fused softmax (exp in place
with accumulated sum, probs pre-scaled by 1/norm so the A.V drain is a
plain copy), A.V with per-batch M=2 probsT columns vs V tiles (N=256),
new-token fixup as one extra rank-8 matmul per batch, out-proj partial
(lhsT = A^T tiles [128,8]). Host sums the 8 partial outputs.
"""

import sys

sys.path.insert(0, "/opt/trn_rl_repo")

import numpy as np
import ml_dtypes

B, S, C = 8, 1, 4096
DIM = 3072
HQ, HKV, HD = 16, 8, 256
NCORES = 8
SCALE = HD ** (-0.5)
BF = ml_dtypes.bfloat16

# packed f32 constant-block column offsets
_CS, _MKV, _DUPA, _DUPB, _IDF, _ONES = 0, 512, 513, 521, 537, 553
_CSTW = 561


def build_bass():
    import concourse.bass as bass  # noqa: F401
    import concourse.mybir as mybir
    import concourse.tile as tile
    from concourse import bacc
    from contextlib import ExitStack

    f32 = mybir.dt.float32
    bf16 = mybir.dt.bfloat16
    Alu = mybir.AluOpType
    Act = mybir.ActivationFunctionType

    nc = bacc.Bacc("TRN2", target_bir_lowering=False, debug=False,
                   num_devices=NCORES)

    xT = nc.dram_tensor("xT", [128, 24 * B], bf16, kind="ExternalInput").ap()
    wq = nc.dram_tensor("wq", [6, 128, 4096], bf16, kind="ExternalInput").ap()
    kt = nc.dram_tensor("kt", [8, 128, 8192], bf16, kind="ExternalInput").ap()
    fmb = nc.dram_tensor("fmb", [1, C], bf16, kind="ExternalInput").ap()
    vt = nc.dram_tensor("vt", [B, 128, 8192], bf16, kind="ExternalInput").ap()
    wo = nc.dram_tensor("wo", [128, 4 * DIM], bf16, kind="ExternalInput").ap()
    cst = nc.dram_tensor("cst", [16, _CSTW], f32, kind="ExternalInput").ap()
    cmk = nc.dram_tensor("cmk", [128, 160], bf16, kind="ExternalInput").ap()
    y = nc.dram_tensor("y", [B, DIM], f32, kind="ExternalOutput").ap()

    with tile.TileContext(nc) as tc, ExitStack() as stk:
        sb = stk.enter_context(tc.tile_pool(name="sb", bufs=1))
        wqp = stk.enter_context(tc.tile_pool(name="wqp", bufs=2))
        kp = stk.enter_context(tc.tile_pool(name="kp", bufs=3))
        vp = stk.enter_context(tc.tile_pool(name="vp", bufs=3))
        tmp = stk.enter_context(tc.tile_pool(name="tmp", bufs=2))
        ps = stk.enter_context(tc.tile_pool(name="ps", bufs=8, space="PSUM"))

        # ---- early small DMAs ----
        xT_sb = sb.tile([128, 24 * B], bf16, tag="xT")
        nc.sync.dma_start(xT_sb[:], xT)
        cst_sb = sb.tile([16, _CSTW], f32, tag="cst")
        nc.sync.dma_start(cst_sb[:], cst)
        cmk_sb = sb.tile([128, 160], bf16, tag="cmk")
        nc.sync.dma_start(cmk_sb[:], cmk)
        fm_sb = sb.tile([1, C], bf16, tag="fm")
        nc.sync.dma_start(fm_sb[:], fmb)
        cmask = cmk_sb[:, 0:128]
        idb16 = cmk_sb[0:16, 128:144]
        idb8 = cmk_sb[0:8, 128:136]
        idb2 = cmk_sb[0:2, 128:130]

        fm = fm_sb[:]
        ones116 = cmk_sb[0:1, 144:160]
        cosq = cst_sb[0:8, _CS:_CS + 128]
        sinq = cst_sb[0:8, _CS + 128:_CS + 256]
        cosk = cst_sb[0:8, _CS + 256:_CS + 384]
        sink = cst_sb[0:8, _CS + 384:_CS + 512]
        mkv = cst_sb[:, _MKV:_MKV + 1]
        dupA = cst_sb[:, _DUPA:_DUPA + 8]
        dupB = cst_sb[0:8, _DUPB:_DUPB + 16]
        idf = cst_sb[:, _IDF:_IDF + 16]
        ones18 = cst_sb[0:1, _ONES:_ONES + 8]

        # ---- phase 1: qkv = x @ Wq_shard; 4 concurrent PE col-groups ----
        pq0 = ps.tile([128, 512], f32, tag="ps", name="pq0")
        pq1 = ps.tile([128, 512], f32, tag="ps", name="pq1")
        for g in range(6):
            wt = wqp.tile([128, 4096], bf16, tag="wq")
            nc.sync.dma_start(wt[:], wq[g])
            for sub in range(4):
                t = 4 * g + sub
                jj = t % 4
                lhs = xT_sb[:, t * 8:(t + 1) * 8]
                nc.tensor.matmul(pq0[32 * jj:32 * jj + 8, :], lhs,
                                 wt[:, sub * 1024:sub * 1024 + 512],
                                 start=(t < 4), stop=(t >= 20),
                                 tile_position=(0, 32 * jj))
                nc.tensor.matmul(pq1[32 * jj:32 * jj + 8, :], lhs,
                                 wt[:, sub * 1024 + 512:sub * 1024 + 1024],
                                 start=(t < 4), stop=(t >= 20),
                                 tile_position=(0, 32 * jj))
        # combine the 4 group partials -> SBUF [8, 512] each
        q_sb = sb.tile([8, 512], f32, tag="q_sb")
        kv_sb = sb.tile([8, 512], f32, tag="kv_sb")
        for dst, src_ps in ((q_sb, pq0), (kv_sb, pq1)):
            nc.vector.tensor_copy(dst[:], src_ps[0:8, :])
            for base in (32, 64, 96):
                nc.vector.tensor_tensor(dst[:], dst[:],
                                        src_ps[base:base + 8, :], op=Alu.add)

        # ---- rope (DVE) on [8, 128] slices; outputs bf16 ----
        qrope = sb.tile([8, 512], bf16, tag="qrope")   # cols (r, half, p)
        krope = sb.tile([8, 256], bf16, tag="krope")   # cols (half, p)
        vnew = sb.tile([8, 256], bf16, tag="vnew")

        def rope(c1, c2, cosa, sina, out1, out2):
            ta = tmp.tile([8, 128], f32, tag="rt", name="ta")
            tb = tmp.tile([8, 128], f32, tag="rt", name="tb")
            nc.vector.tensor_tensor(ta[:], c1, cosa, op=Alu.mult)
            nc.vector.tensor_tensor(tb[:], c2, sina, op=Alu.mult)
            nc.vector.tensor_tensor(out1, ta[:], tb[:], op=Alu.subtract)
            tc_ = tmp.tile([8, 128], f32, tag="rt", name="tc_")
            td = tmp.tile([8, 128], f32, tag="rt", name="td")
            nc.vector.tensor_tensor(tc_[:], c1, sina, op=Alu.mult)
            nc.vector.tensor_tensor(td[:], c2, cosa, op=Alu.mult)
            nc.vector.tensor_tensor(out2, tc_[:], td[:], op=Alu.add)

        for r in range(2):
            rope(q_sb[:, r * 256:r * 256 + 128],
                 q_sb[:, r * 256 + 128:(r + 1) * 256],
                 cosq, sinq,
                 qrope[:, (2 * r) * 128:(2 * r) * 128 + 128],
                 qrope[:, (2 * r + 1) * 128:(2 * r + 1) * 128 + 128])
        rope(kv_sb[:, 0:128], kv_sb[:, 128:256], cosk, sink,
             krope[:, 0:128], krope[:, 128:256])
        nc.scalar.copy(vnew[:], kv_sb[:, 256:512])

        # ---- transposes to qT halves [128,16] (cols 2b+r) and kT [128,8] ----
        qTh = [sb.tile([128, 16], bf16, tag=f"qTh{h}", name=f"qTh{h}")
               for h in range(2)]
        knT = [sb.tile([128, 8], bf16, tag=f"knT{h}", name=f"knT{h}")
               for h in range(2)]
        for r in range(2):
            for h in range(2):
                pt = ps.tile([128, 8], bf16, tag="ps", name=f"ptq{r}{h}")
                nc.tensor.transpose(pt[:], qrope[:, (2 * r + h) * 128:
                                                  (2 * r + h + 1) * 128],
                                    idb8)
                dst = qTh[h][:].rearrange("p (b r) -> p r b", r=2)[:, r]
                nc.scalar.copy(dst, pt[:])
        for h in range(2):
            pt = ps.tile([128, 8], bf16, tag="ps", name=f"ptk{h}")
            nc.tensor.transpose(pt[:], krope[:, h * 128:(h + 1) * 128],
                                idb8)
            nc.scalar.copy(knT[h][:], pt[:])

        # ---- s_new[16,1]: q . k_new, diag extraction ----
        psn = ps.tile([16, 8], f32, tag="ps", name="psn")
        for h in range(2):
            nc.tensor.matmul(psn[:], qTh[h][:], knT[h][:],
                             start=(h == 0), stop=(h == 1))
        snm = sb.tile([16, 8], f32, tag="snm")
        nc.vector.tensor_tensor(snm[:], psn[:], dupA, op=Alu.mult)
        s_new = sb.tile([16, 1], f32, tag="snew")
        nc.vector.tensor_reduce(s_new[:], snm[:], axis=mybir.AxisListType.X,
                                op=Alu.add)
        nc.vector.tensor_scalar_add(s_new[:], s_new[:], mkv)

        # masked q: qThM[h][:, b*16+c] = qTh[h][:, c] if c in {2b, 2b+1} else 0
        qThM = [sb.tile([128, 128], bf16, tag=f"qThM{h}", name=f"qThM{h}")
                for h in range(2)]
        for h in range(2):
            for b in range(B):
                nc.vector.tensor_tensor(qThM[h][:, b * 16:(b + 1) * 16],
                                        qTh[h][:],
                                        cmask[:, b * 16:(b + 1) * 16],
                                        op=Alu.mult)

        # ---- phase 2: scores [16, 4096]; kt tiles are chunk-major, the 16
        # (b, h) matmuls per chunk run in 4 concurrent PE col-groups, and the
        # fm mask row lands via a rank-1 matmul in group 0 ----
        scores = sb.tile([16, C], f32, tag="scores")
        mparts = sb.tile([16, 8], f32, tag="mparts")
        for j in range(8):
            ktile = kp.tile([128, 8192], bf16, tag="k", name=f"k{j}")
            nc.sync.dma_start(ktile[:], kt[j])
            pch = ps.tile([128, 512], f32, tag="ps", name=f"sc{j}")
            ssl = slice(j * 512, (j + 1) * 512)
            nc.tensor.matmul(pch[0:16, :], ones116, fm[:, ssl],
                             start=True, stop=False, tile_position=(0, 0))
            for b in range(B):
                jj = b % 4
                out = pch[32 * jj:32 * jj + 16, :]
                nc.tensor.matmul(out, qThM[0][:, b * 16:(b + 1) * 16],
                                 ktile[:, b * 1024:b * 1024 + 512],
                                 start=(b in (1, 2, 3)), stop=False,
                                 tile_position=(0, 32 * jj))
                nc.tensor.matmul(out, qThM[1][:, b * 16:(b + 1) * 16],
                                 ktile[:, b * 1024 + 512:(b + 1) * 1024],
                                 start=False, stop=(b >= 4),
                                 tile_position=(0, 32 * jj))
            nc.vector.tensor_copy(scores[:, ssl], pch[0:16, :])
            for base in (32, 64, 96):
                nc.vector.tensor_tensor(scores[:, ssl], scores[:, ssl],
                                        pch[base:base + 16, :], op=Alu.add)
            nc.vector.tensor_reduce(mparts[:, j:j + 1], scores[:, ssl],
                                    axis=mybir.AxisListType.X, op=Alu.max)

        # ---- softmax (new token via s_new; probs pre-scaled by 1/norm) ----
        m1 = sb.tile([16, 1], f32, tag="m1")
        nc.vector.tensor_reduce(m1[:], mparts[:], axis=mybir.AxisListType.X,
                                op=Alu.max)
        tmax = sb.tile([16, 1], f32, tag="tmax")
        nc.vector.tensor_tensor(tmax[:], m1[:], s_new[:], op=Alu.max)
        negmax = sb.tile([16, 1], f32, tag="negmax")
        nc.vector.tensor_scalar_mul(negmax[:], tmax[:], -1.0)
        sumz = sb.tile([16, 1], f32, tag="sumz")
        nc.scalar.activation(scores[:], scores[:], Act.Exp, bias=negmax[:],
                             accum_out=sumz[:])
        p_new = sb.tile([16, 1], f32, tag="pnew")
        nc.scalar.activation(p_new[:], s_new[:], Act.Exp, bias=negmax[:])
        norm = sb.tile([16, 1], f32, tag="norm")
        nc.vector.tensor_tensor(norm[:], sumz[:], p_new[:], op=Alu.add)
        rnorm = sb.tile([16, 1], f32, tag="rnorm")
        nc.vector.reciprocal(rnorm[:], norm[:])
        probs = sb.tile([16, C], bf16, tag="probs")
        nc.vector.tensor_scalar_mul(probs[:], scores[:], rnorm[:, 0:1])
        pnorm = sb.tile([16, 1], f32, tag="pnorm")
        nc.vector.tensor_tensor(pnorm[:], p_new[:], rnorm[:], op=Alu.mult)

        # probsT via PE transpose: 32 x [16,128] -> [128,16] bf16
        probsT = sb.tile([128, 32 * 16], bf16, tag="probsT")
        for ct in range(32):
            pt = ps.tile([128, 16], bf16, tag="ps", name=f"pt{ct}")
            nc.tensor.transpose(pt[:], probs[:, ct * 128:(ct + 1) * 128],
                                idb16)
            nc.scalar.copy(probsT[:, ct * 16:(ct + 1) * 16], pt[:])

        # selPT[8,16] = dupB * broadcast(pnorm^T): for new-token A.V term
        pnt = ps.tile([1, 16], f32, tag="ps", name="pnt")
        nc.tensor.transpose(pnt[:], pnorm[:], idf)
        pnT = sb.tile([1, 16], f32, tag="pnT")
        nc.scalar.copy(pnT[:], pnt[:])
        pb = ps.tile([8, 16], f32, tag="ps", name="pb")
        nc.tensor.matmul(pb[:], ones18, pnT[:], start=True, stop=True)
        selPT = sb.tile([8, 16], bf16, tag="selPT")
        nc.vector.tensor_tensor(selPT[:], pb[:], dupB, op=Alu.mult)

        # ---- phase 3: A = probs @ V per batch, M=2, N=256 ----
        wo_sb = sb.tile([128, 4 * DIM], bf16, tag="wo")
        aTall = sb.tile([128, 32], bf16, tag="aTall")  # cols (r, half, b)
        for b in range(B):
            vtile = vp.tile([128, 8192], bf16, tag="v", name=f"v{b}")
            nc.sync.dma_start(vtile[:], vt[b])
            if b == 3:
                nc.sync.dma_start(wo_sb[:], wo)
            pav = ps.tile([128, 256], f32, tag="ps", name=f"av{b}")
            for ct in range(32):
                jj = ct % 4
                nc.tensor.matmul(pav[32 * jj:32 * jj + 2, :],
                                 probsT[:, ct * 16 + 2 * b:ct * 16 + 2 * b + 2],
                                 vtile[:, ct * 256:(ct + 1) * 256],
                                 start=(ct < 4), stop=(ct >= 28 and jj != 0),
                                 tile_position=(0, 32 * jj))
            nc.tensor.matmul(pav[0:2, :], selPT[:, 2 * b:2 * b + 2], vnew[:],
                             start=False, stop=True, tile_position=(0, 0))
            af = tmp.tile([2, 256], f32, tag="adr", name=f"af{b}")
            nc.vector.tensor_copy(af[:], pav[0:2, :])
            for base in (32, 64):
                nc.vector.tensor_tensor(af[:], af[:],
                                        pav[base:base + 2, :], op=Alu.add)
            asb = tmp.tile([2, 256], bf16, tag="asb", name=f"asb{b}")
            nc.vector.tensor_tensor(asb[:], af[:], pav[96:98, :], op=Alu.add)
            for h in range(2):
                pt2 = ps.tile([128, 2], bf16, tag="ps", name=f"pat{b}{h}")
                nc.tensor.transpose(pt2[:], asb[:, h * 128:(h + 1) * 128],
                                    idb2)
                dst = aTall[:].rearrange("p (r h b) -> p h b r", r=2, h=2,
                                         b=8)[:, h, b]
                nc.scalar.copy(dst, pt2[:])

        # ---- phase 4: y = A^T tiles @ Wout_shard ----
        y_sb = sb.tile([B, DIM], f32, tag="ysb")
        pys = [ps.tile([8, 512], f32, tag="ps", name=f"py{n}")
               for n in range(6)]
        for t in range(4):
            for nch in range(6):
                nc.tensor.matmul(pys[nch][:], aTall[:, t * 8:(t + 1) * 8],
                                 wo_sb[:, t * DIM + nch * 512:
                                       t * DIM + (nch + 1) * 512],
                                 start=(t == 0), stop=(t == 3))
        for nch in range(6):
            nc.vector.tensor_copy(y_sb[:, nch * 512:(nch + 1) * 512],
                                  pys[nch][:])
        nc.sync.dma_start(y, y_sb[:])

    nc.compile()
    return nc


_CACHED = {}


def _get_bass():
    if "nc" not in _CACHED:
        _CACHED["nc"] = build_bass()
    return _CACHED["nc"]


def _prep_inputs(x, freqs_cos, freqs_sin, kv, k_cache, v_cache, mask,
                 W_qkv, W_out):
    x2 = np.asarray(x, np.float32).reshape(B, DIM)
    xT192 = np.ascontiguousarray(
        x2.T.reshape(24, 128, B).transpose(1, 0, 2).reshape(128, 24 * B)
    ).astype(BF)
    cos = np.asarray(freqs_cos, np.float32)[0]
    sin = np.asarray(freqs_sin, np.float32)[0]
    kvp = int(np.asarray(kv).reshape(-1)[0])
    maskr = np.asarray(mask, np.float32)

    cst = np.zeros((16, _CSTW), np.float32)
    fmb = maskr[0:1].astype(np.float32).copy()
    fmb[0, kvp] -= 1e30
    fmb = fmb.astype(BF)
    cs = np.concatenate([cos * SCALE, sin * SCALE, cos, sin])
    cst[0:8, _CS:_CS + 512] = np.tile(cs, (8, 1))
    cst[:, _MKV] = maskr[0, kvp]
    for b in range(B):
        cst[2 * b, _DUPA + b] = 1.0
        cst[2 * b + 1, _DUPA + b] = 1.0
        cst[b, _DUPB + 2 * b] = 1.0
        cst[b, _DUPB + 2 * b + 1] = 1.0
    cst[:, _IDF:_IDF + 16] = np.eye(16, dtype=np.float32)
    cst[0, _ONES:_ONES + 8] = 1.0
    cmk = np.zeros((128, 160), np.float32)
    for b in range(B):
        cmk[:, b * 16 + 2 * b] = 1.0
        cmk[:, b * 16 + 2 * b + 1] = 1.0
    cmk[0:16, 128:144] = np.eye(16, dtype=np.float32)
    cmk[0, 144:160] = 1.0
    cmk = cmk.astype(BF)

    KB = np.asarray(k_cache, np.float32).astype(BF)   # [B, C, HKV, HD]
    VB = np.asarray(v_cache, np.float32).astype(BF)
    WqB = np.asarray(W_qkv, np.float32).astype(BF)    # [DIM, 8192]
    WoB = np.asarray(W_out, np.float32).astype(BF)    # [4096, DIM]

    in_maps = []
    for m in range(NCORES):
        wq_shard = np.concatenate([
            WqB[:, 2 * m * HD:(2 * m + 2) * HD],
            WqB[:, HQ * HD + m * HD: HQ * HD + (m + 1) * HD],
            WqB[:, (HQ + HKV) * HD + m * HD: (HQ + HKV) * HD + (m + 1) * HD],
        ], axis=1)                                     # [3072, 1024]
        wq6 = np.ascontiguousarray(
            wq_shard.reshape(6, 4, 128, 1024).transpose(0, 2, 1, 3)
        ).reshape(6, 128, 4096)
        kts = np.ascontiguousarray(
            KB[:, :, m, :].reshape(B, 8, 512, 2, 128)
            .transpose(1, 4, 0, 3, 2)
        ).reshape(8, 128, 8192)
        vts = np.ascontiguousarray(
            VB[:, :, m, :].reshape(B, 32, 128, HD).transpose(0, 2, 1, 3)
        ).reshape(B, 128, 8192)
        wo4 = np.ascontiguousarray(
            WoB[2 * m * HD:(2 * m + 2) * HD, :].reshape(2, 2, 128, DIM)
            .transpose(2, 0, 1, 3)
        ).reshape(128, 4 * DIM)
        in_maps.append({
            "xT": xT192, "wq": wq6, "kt": kts, "vt": vts, "wo": wo4,
            "cst": cst, "cmk": cmk, "fmb": fmb,
        })
    return in_maps


def _run(inputs, trace=False):
    from concourse.bass_utils import run_bass_kernel_spmd
    nc = _get_bass()
    in_maps = _prep_inputs(**inputs)
    res = run_bass_kernel_spmd(nc, in_maps, core_ids=list(range(NCORES)),
                               trace=trace)
    parts = [r["y"] for r in res.results]
    out = np.sum(np.stack(parts, 0), 0, dtype=np.float32)
    return out.reshape(B, S, DIM), res


def kernel(**inputs):
    out, _ = _run(inputs, trace=False)
    return out


# revision 22
# speedup vs baseline: 1.4422x; 1.1436x over previous
"""TP-8 decode attention kernel for TRN2 (Bass/Tile), bf16 compute.

Shards the 8 KV heads (2 q heads each) across 8 NeuronCores. Host
pre-casts weights and KV cache to bf16 (HBM traffic 86.5 -> 43.3 MB per
core) and lays every tensor out in the exact tile order the kernel
consumes, so all big DMAs are contiguous and >= 1 MiB.

Per core: qkv projection (lhsT = xT k-tiles [128,8], rhs = Wq bf16
streamed N=512), RoPE on DVE, PE transposes to head-major qT/kT, scores
with q stationary ([128,16] lhsT, K streamed N=512, all-batch rows with
per-batch extraction at the PSUM drain), fused softmax (exp in place
with accumulated sum, probs pre-scaled by 1/norm so the A.V drain is a
plain copy), A.V with per-batch M=2 probsT columns vs V tiles (N=256),
new-token fixup as one extra rank-8 matmul per batch, out-proj partial
(lhsT = A^T tiles [128,8]). Host sums the 8 partial outputs.
"""

import sys

sys.path.insert(0, "/opt/trn_rl_repo")

import numpy as np
import ml_dtypes

B, S, C = 8, 1, 4096
DIM = 3072
HQ, HKV, HD = 16, 8, 256
NCORES = 8
SCALE = HD ** (-0.5)
BF = ml_dtypes.bfloat16

# packed f32 constant-block column offsets
_CS, _MKV, _DUPA, _DUPB, _IDF, _ONES = 0, 512, 513, 521, 537, 553
_CSTW = 561


def build_bass():
    import concourse.bass as bass  # noqa: F401
    import concourse.mybir as mybir
    import concourse.tile as tile
    from concourse import bacc
    from contextlib import ExitStack

    f32 = mybir.dt.float32
    bf16 = mybir.dt.bfloat16
    Alu = mybir.AluOpType
    Act = mybir.ActivationFunctionType

    nc = bacc.Bacc("TRN2", target_bir_lowering=False, debug=False,
                   num_devices=NCORES)

    xT = nc.dram_tensor("xT", [128, 24 * B], bf16, kind="ExternalInput").ap()
    wq = nc.dram_tensor("wq", [6, 128, 4096], bf16, kind="ExternalInput").ap()
    kt = nc.dram_tensor("kt", [8, 128, 8192], bf16, kind="ExternalInput").ap()
    fmb = nc.dram_tensor("fmb", [1, C], bf16, kind="ExternalInput").ap()
    vt = nc.dram_tensor("vt", [B, 128, 8192], bf16, kind="ExternalInput").ap()
    wo = nc.dram_tensor("wo", [128, 4 * DIM], bf16, kind="ExternalInput").ap()
    cst = nc.dram_tensor("cst", [16, _CSTW], f32, kind="ExternalInput").ap()
    cmk = nc.dram_tensor("cmk", [128, 160], bf16, kind="ExternalInput").ap()
    y = nc.dram_tensor("y", [B, DIM], f32, kind="ExternalOutput").ap()

    with tile.TileContext(nc) as tc, ExitStack() as stk:
        sb = stk.enter_context(tc.tile_pool(name="sb", bufs=1))
        wqp = stk.enter_context(tc.tile_pool(name="wqp", bufs=3))
        kp = stk.enter_context(tc.tile_pool(name="kp", bufs=3))
        vp = stk.enter_context(tc.tile_pool(name="vp", bufs=3))
        tmp = stk.enter_context(tc.tile_pool(name="tmp", bufs=2))
        ps = stk.enter_context(tc.tile_pool(name="ps", bufs=8, space="PSUM"))

        # ---- DMA order: xT + all wq tiles first (qkv is the first
        # consumer), then the small constant blocks ----
        xT_sb = sb.tile([128, 24 * B], bf16, tag="xT")
        nc.sync.dma_start(xT_sb[:], xT)
        wts = []
        for g in range(6):
            wt = wqp.tile([128, 4096], bf16, tag="wq", name=f"wq{g}")
            nc.sync.dma_start(wt[:], wq[g])
            wts.append(wt)
        cst_sb = sb.tile([16, _CSTW], f32, tag="cst")
        nc.sync.dma_start(cst_sb[:], cst)
        cmk_sb = sb.tile([128, 160], bf16, tag="cmk")
        nc.sync.dma_start(cmk_sb[:], cmk)
        fm_sb = sb.tile([1, C], bf16, tag="fm")
        nc.sync.dma_start(fm_sb[:], fmb)
        cmask = cmk_sb[:, 0:128]
        idb16 = cmk_sb[0:16, 128:144]
        idb8 = cmk_sb[0:8, 128:136]
        idb2 = cmk_sb[0:2, 128:130]

        fm = fm_sb[:]
        ones116 = cmk_sb[0:1, 144:160]
        cosq = cst_sb[0:8, _CS:_CS + 128]
        sinq = cst_sb[0:8, _CS + 128:_CS + 256]
        cosk = cst_sb[0:8, _CS + 256:_CS + 384]
        sink = cst_sb[0:8, _CS + 384:_CS + 512]
        mkv = cst_sb[:, _MKV:_MKV + 1]
        dupA = cst_sb[:, _DUPA:_DUPA + 8]
        dupB = cst_sb[0:8, _DUPB:_DUPB + 16]
        idf = cst_sb[:, _IDF:_IDF + 16]
        ones18 = cst_sb[0:1, _ONES:_ONES + 8]

        # ---- phase 1: qkv = x @ Wq_shard; 4 concurrent PE col-groups ----
        pq0 = ps.tile([128, 512], f32, tag="ps", name="pq0")
        pq1 = ps.tile([128, 512], f32, tag="ps", name="pq1")
        for g in range(6):
            wt = wts[g]
            for sub in range(4):
                t = 4 * g + sub
                jj = t % 4
                lhs = xT_sb[:, t * 8:(t + 1) * 8]
                nc.tensor.matmul(pq0[32 * jj:32 * jj + 8, :], lhs,
                                 wt[:, sub * 1024:sub * 1024 + 512],
                                 start=(t < 4), stop=(t >= 20),
                                 tile_position=(0, 32 * jj))
                nc.tensor.matmul(pq1[32 * jj:32 * jj + 8, :], lhs,
                                 wt[:, sub * 1024 + 512:sub * 1024 + 1024],
                                 start=(t < 4), stop=(t >= 20),
                                 tile_position=(0, 32 * jj))
        # combine the 4 group partials -> SBUF [8, 512] each
        q_sb = sb.tile([8, 512], f32, tag="q_sb")
        kv_sb = sb.tile([8, 512], f32, tag="kv_sb")
        for dst, src_ps in ((q_sb, pq0), (kv_sb, pq1)):
            nc.vector.tensor_copy(dst[:], src_ps[0:8, :])
            for base in (32, 64, 96):
                nc.vector.tensor_tensor(dst[:], dst[:],
                                        src_ps[base:base + 8, :], op=Alu.add)

        # ---- rope (DVE) on [8, 128] slices; outputs bf16 ----
        qrope = sb.tile([8, 512], bf16, tag="qrope")   # cols (r, half, p)
        krope = sb.tile([8, 256], bf16, tag="krope")   # cols (half, p)
        vnew = sb.tile([8, 256], bf16, tag="vnew")

        def rope(c1, c2, cosa, sina, out1, out2):
            ta = tmp.tile([8, 128], f32, tag="rt", name="ta")
            tb = tmp.tile([8, 128], f32, tag="rt", name="tb")
            nc.vector.tensor_tensor(ta[:], c1, cosa, op=Alu.mult)
            nc.vector.tensor_tensor(tb[:], c2, sina, op=Alu.mult)
            nc.vector.tensor_tensor(out1, ta[:], tb[:], op=Alu.subtract)
            tc_ = tmp.tile([8, 128], f32, tag="rt", name="tc_")
            td = tmp.tile([8, 128], f32, tag="rt", name="td")
            nc.vector.tensor_tensor(tc_[:], c1, sina, op=Alu.mult)
            nc.vector.tensor_tensor(td[:], c2, cosa, op=Alu.mult)
            nc.vector.tensor_tensor(out2, tc_[:], td[:], op=Alu.add)

        for r in range(2):
            rope(q_sb[:, r * 256:r * 256 + 128],
                 q_sb[:, r * 256 + 128:(r + 1) * 256],
                 cosq, sinq,
                 qrope[:, (2 * r) * 128:(2 * r) * 128 + 128],
                 qrope[:, (2 * r + 1) * 128:(2 * r + 1) * 128 + 128])
        rope(kv_sb[:, 0:128], kv_sb[:, 128:256], cosk, sink,
             krope[:, 0:128], krope[:, 128:256])
        nc.scalar.copy(vnew[:], kv_sb[:, 256:512])

        # ---- transposes to qT halves [128,16] (cols 2b+r) and kT [128,8] ----
        qTh = [sb.tile([128, 16], bf16, tag=f"qTh{h}", name=f"qTh{h}")
               for h in range(2)]
        knT = [sb.tile([128, 8], bf16, tag=f"knT{h}", name=f"knT{h}")
               for h in range(2)]
        for r in range(2):
            for h in range(2):
                pt = ps.tile([128, 8], bf16, tag="ps", name=f"ptq{r}{h}")
                nc.tensor.transpose(pt[:], qrope[:, (2 * r + h) * 128:
                                                  (2 * r + h + 1) * 128],
                                    idb8)
                dst = qTh[h][:].rearrange("p (b r) -> p r b", r=2)[:, r]
                nc.vector.tensor_copy(dst, pt[:])
        for h in range(2):
            pt = ps.tile([128, 8], bf16, tag="ps", name=f"ptk{h}")
            nc.tensor.transpose(pt[:], krope[:, h * 128:(h + 1) * 128],
                                idb8)
            nc.vector.tensor_copy(knT[h][:], pt[:])

        # ---- s_new[16,1]: q . k_new, diag extraction ----
        psn = ps.tile([16, 8], f32, tag="ps", name="psn")
        for h in range(2):
            nc.tensor.matmul(psn[:], qTh[h][:], knT[h][:],
                             start=(h == 0), stop=(h == 1))
        snm = sb.tile([16, 8], f32, tag="snm")
        nc.vector.tensor_tensor(snm[:], psn[:], dupA, op=Alu.mult)
        s_new = sb.tile([16, 1], f32, tag="snew")
        nc.vector.tensor_reduce(s_new[:], snm[:], axis=mybir.AxisListType.X,
                                op=Alu.add)
        nc.vector.tensor_scalar_add(s_new[:], s_new[:], mkv)

        # masked q: qThM[h][:, b*16+c] = qTh[h][:, c] if c in {2b, 2b+1} else 0
        qThM = [sb.tile([128, 128], bf16, tag=f"qThM{h}", name=f"qThM{h}")
                for h in range(2)]
        for h in range(2):
            for b in range(B):
                nc.vector.tensor_tensor(qThM[h][:, b * 16:(b + 1) * 16],
                                        qTh[h][:],
                                        cmask[:, b * 16:(b + 1) * 16],
                                        op=Alu.mult)

        # ---- phase 2: scores [16, 4096]; kt tiles are chunk-major, the 16
        # (b, h) matmuls per chunk run in 4 concurrent PE col-groups, and the
        # fm mask row lands via a rank-1 matmul in group 0 ----
        scores = sb.tile([16, C], f32, tag="scores")
        mparts = sb.tile([16, 8], f32, tag="mparts")
        for j in range(8):
            ktile = kp.tile([128, 8192], bf16, tag="k", name=f"k{j}")
            nc.sync.dma_start(ktile[:], kt[j])
            pch = ps.tile([128, 512], f32, tag="ps", name=f"sc{j}")
            ssl = slice(j * 512, (j + 1) * 512)
            nc.tensor.matmul(pch[0:16, :], ones116, fm[:, ssl],
                             start=True, stop=False, tile_position=(0, 0))
            for b in range(B):
                jj = b % 4
                out = pch[32 * jj:32 * jj + 16, :]
                nc.tensor.matmul(out, qThM[0][:, b * 16:(b + 1) * 16],
                                 ktile[:, b * 1024:b * 1024 + 512],
                                 start=(b in (1, 2, 3)), stop=False,
                                 tile_position=(0, 32 * jj))
                nc.tensor.matmul(out, qThM[1][:, b * 16:(b + 1) * 16],
                                 ktile[:, b * 1024 + 512:(b + 1) * 1024],
                                 start=False, stop=(b >= 4),
                                 tile_position=(0, 32 * jj))
            nc.vector.tensor_copy(scores[:, ssl], pch[0:16, :])
            for base in (32, 64, 96):
                nc.vector.tensor_tensor(scores[:, ssl], scores[:, ssl],
                                        pch[base:base + 16, :], op=Alu.add)
            nc.vector.tensor_reduce(mparts[:, j:j + 1], scores[:, ssl],
                                    axis=mybir.AxisListType.X, op=Alu.max)

        # ---- softmax (new token via s_new; probs pre-scaled by 1/norm) ----
        m1 = sb.tile([16, 1], f32, tag="m1")
        nc.vector.tensor_reduce(m1[:], mparts[:], axis=mybir.AxisListType.X,
                                op=Alu.max)
        tmax = sb.tile([16, 1], f32, tag="tmax")
        nc.vector.tensor_tensor(tmax[:], m1[:], s_new[:], op=Alu.max)
        negmax = sb.tile([16, 1], f32, tag="negmax")
        nc.vector.tensor_scalar_mul(negmax[:], tmax[:], -1.0)
        sumz = sb.tile([16, 1], f32, tag="sumz")
        nc.scalar.activation(scores[:], scores[:], Act.Exp, bias=negmax[:],
                             accum_out=sumz[:])
        p_new = sb.tile([16, 1], f32, tag="pnew")
        nc.scalar.activation(p_new[:], s_new[:], Act.Exp, bias=negmax[:])
        norm = sb.tile([16, 1], f32, tag="norm")
        nc.vector.tensor_tensor(norm[:], sumz[:], p_new[:], op=Alu.add)
        rnorm = sb.tile([16, 1], f32, tag="rnorm")
        nc.vector.reciprocal(rnorm[:], norm[:])
        probs = sb.tile([16, C], bf16, tag="probs")
        nc.vector.tensor_scalar_mul(probs[:], scores[:], rnorm[:, 0:1])
        pnorm = sb.tile([16, 1], f32, tag="pnorm")
        nc.vector.tensor_tensor(pnorm[:], p_new[:], rnorm[:], op=Alu.mult)

        # probsT via PE transpose: 32 x [16,128] -> [128,16] bf16
        probsT = sb.tile([128, 32 * 16], bf16, tag="probsT")
        for ct in range(32):
            pt = ps.tile([128, 16], bf16, tag="ps", name=f"pt{ct}")
            nc.tensor.transpose(pt[:], probs[:, ct * 128:(ct + 1) * 128],
                                idb16)
            nc.vector.tensor_copy(probsT[:, ct * 16:(ct + 1) * 16], pt[:])

        # selPT[8,16] = dupB * broadcast(pnorm^T): for new-token A.V term
        pnt = ps.tile([1, 16], f32, tag="ps", name="pnt")
        nc.tensor.transpose(pnt[:], pnorm[:], idf)
        pnT = sb.tile([1, 16], f32, tag="pnT")
        nc.scalar.copy(pnT[:], pnt[:])
        pb = ps.tile([8, 16], f32, tag="ps", name="pb")
        nc.tensor.matmul(pb[:], ones18, pnT[:], start=True, stop=True)
        selPT = sb.tile([8, 16], bf16, tag="selPT")
        nc.vector.tensor_tensor(selPT[:], pb[:], dupB, op=Alu.mult)

        # ---- phase 3: A = probs @ V per batch, M=2, N=256 ----
        wo_sb = sb.tile([128, 4 * DIM], bf16, tag="wo")
        aTall = sb.tile([128, 32], bf16, tag="aTall")  # cols (r, half, b)
        for b in range(B):
            vtile = vp.tile([128, 8192], bf16, tag="v", name=f"v{b}")
            nc.sync.dma_start(vtile[:], vt[b])
            if b == 3:
                nc.sync.dma_start(wo_sb[:], wo)
            pav = ps.tile([128, 256], f32, tag="ps", name=f"av{b}")
            for ct in range(32):
                jj = ct % 4
                nc.tensor.matmul(pav[32 * jj:32 * jj + 2, :],
                                 probsT[:, ct * 16 + 2 * b:ct * 16 + 2 * b + 2],
                                 vtile[:, ct * 256:(ct + 1) * 256],
                                 start=(ct < 4), stop=(ct >= 28 and jj != 0),
                                 tile_position=(0, 32 * jj))
            nc.tensor.matmul(pav[0:2, :], selPT[:, 2 * b:2 * b + 2], vnew[:],
                             start=False, stop=True, tile_position=(0, 0))
            af = tmp.tile([2, 256], f32, tag="adr", name=f"af{b}")
            nc.vector.tensor_copy(af[:], pav[0:2, :])
            for base in (32, 64):
                nc.vector.tensor_tensor(af[:], af[:],
                                        pav[base:base + 2, :], op=Alu.add)
            asb = tmp.tile([2, 256], bf16, tag="asb", name=f"asb{b}")
            nc.vector.tensor_tensor(asb[:], af[:], pav[96:98, :], op=Alu.add)
            for h in range(2):
                pt2 = ps.tile([128, 2], bf16, tag="ps", name=f"pat{b}{h}")
                nc.tensor.transpose(pt2[:], asb[:, h * 128:(h + 1) * 128],
                                    idb2)
                dst = aTall[:].rearrange("p (r h b) -> p h b r", r=2, h=2,
                                         b=8)[:, h, b]
                nc.vector.tensor_copy(dst, pt2[:])

        # ---- phase 4: y = A^T tiles @ Wout_shard ----
        y_sb = sb.tile([B, DIM], f32, tag="ysb")
        pys = [ps.tile([8, 512], f32, tag="ps", name=f"py{n}")
               for n in range(6)]
        for t in range(4):
            for nch in range(6):
                nc.tensor.matmul(pys[nch][:], aTall[:, t * 8:(t + 1) * 8],
                                 wo_sb[:, t * DIM + nch * 512:
                                       t * DIM + (nch + 1) * 512],
                                 start=(t == 0), stop=(t == 3))
        for nch in range(6):
            nc.vector.tensor_copy(y_sb[:, nch * 512:(nch + 1) * 512],
                                  pys[nch][:])
        nc.sync.dma_start(y, y_sb[:])

    nc.compile()
    return nc


_CACHED = {}


def _get_bass():
    if "nc" not in _CACHED:
        _CACHED["nc"] = build_bass()
    return _CACHED["nc"]


def _prep_inputs(x, freqs_cos, freqs_sin, kv, k_cache, v_cache, mask,
                 W_qkv, W_out):
    x2 = np.asarray(x, np.float32).reshape(B, DIM)
    xT192 = np.ascontiguousarray(
        x2.T.reshape(24, 128, B).transpose(1, 0, 2).reshape(128, 24 * B)
    ).astype(BF)
    cos = np.asarray(freqs_cos, np.float32)[0]
    sin = np.asarray(freqs_sin, np.float32)[0]
    kvp = int(np.asarray(kv).reshape(-1)[0])
    maskr = np.asarray(mask, np.float32)

    cst = np.zeros((16, _CSTW), np.float32)
    fmb = maskr[0:1].astype(np.float32).copy()
    fmb[0, kvp] -= 1e30
    fmb = fmb.astype(BF)
    cs = np.concatenate([cos * SCALE, sin * SCALE, cos, sin])
    cst[0:8, _CS:_CS + 512] = np.tile(cs, (8, 1))
    cst[:, _MKV] = maskr[0, kvp]
    for b in range(B):
        cst[2 * b, _DUPA + b] = 1.0
        cst[2 * b + 1, _DUPA + b] = 1.0
        cst[b, _DUPB + 2 * b] = 1.0
        cst[b, _DUPB + 2 * b + 1] = 1.0
    cst[:, _IDF:_IDF + 16] = np.eye(16, dtype=np.float32)
    cst[0, _ONES:_ONES + 8] = 1.0
    cmk = np.zeros((128, 160), np.float32)
    for b in range(B):
        cmk[:, b * 16 + 2 * b] = 1.0
        cmk[:, b * 16 + 2 * b + 1] = 1.0
    cmk[0:16, 128:144] = np.eye(16, dtype=np.float32)
    cmk[0, 144:160] = 1.0
    cmk = cmk.astype(BF)

    KB = np.asarray(k_cache, np.float32).astype(BF)   # [B, C, HKV, HD]
    VB = np.asarray(v_cache, np.float32).astype(BF)
    WqB = np.asarray(W_qkv, np.float32).astype(BF)    # [DIM, 8192]
    WoB = np.asarray(W_out, np.float32).astype(BF)    # [4096, DIM]

    in_maps = []
    for m in range(NCORES):
        wq_shard = np.concatenate([
            WqB[:, 2 * m * HD:(2 * m + 2) * HD],
            WqB[:, HQ * HD + m * HD: HQ * HD + (m + 1) * HD],
            WqB[:, (HQ + HKV) * HD + m * HD: (HQ + HKV) * HD + (m + 1) * HD],
        ], axis=1)                                     # [3072, 1024]
        wq6 = np.ascontiguousarray(
            wq_shard.reshape(6, 4, 128, 1024).transpose(0, 2, 1, 3)
        ).reshape(6, 128, 4096)
        kts = np.ascontiguousarray(
            KB[:, :, m, :].reshape(B, 8, 512, 2, 128)
            .transpose(1, 4, 0, 3, 2)
        ).reshape(8, 128, 8192)
        vts = np.ascontiguousarray(
            VB[:, :, m, :].reshape(B, 32, 128, HD).transpose(0, 2, 1, 3)
        ).reshape(B, 128, 8192)
        wo4 = np.ascontiguousarray(
            WoB[2 * m * HD:(2 * m + 2) * HD, :].reshape(2, 2, 128, DIM)
            .transpose(2, 0, 1, 3)
        ).reshape(128, 4 * DIM)
        in_maps.append({
            "xT": xT192, "wq": wq6, "kt": kts, "vt": vts, "wo": wo4,
            "cst": cst, "cmk": cmk, "fmb": fmb,
        })
    return in_maps


def _run(inputs, trace=False):
    from concourse.bass_utils import run_bass_kernel_spmd
    nc = _get_bass()
    in_maps = _prep_inputs(**inputs)
    res = run_bass_kernel_spmd(nc, in_maps, core_ids=list(range(NCORES)),
                               trace=trace)
    parts = [r["y"] for r in res.results]
    out = np.sum(np.stack(parts, 0), 0, dtype=np.float32)
    return out.reshape(B, S, DIM), res


def kernel(**inputs):
    out, _ = _run(inputs, trace=False)
    return out
